# revision 6
# baseline (speedup 1.0000x reference)
"""GQA kernel for Trainium2, 8 NeuronCores.

Problem: B=4, T=2048, E=2048, G=4 kv groups, QG=4 queries/group, D=128.
Sharding: core c handles batch b=c//2 and kv-groups {2*(c%2), 2*(c%2)+1}.
Each core computes a partial o_proj output for its batch (its 2 groups'
contribution); host sums the two partials per batch.

Device layouts (all matmuls bf16 with fp32 PSUM accumulation):
  xT   [128e, 16et, 512t]  per t-chunk, via DMA-transpose of x[b] (bf16)
  qT   [128d, 8h, 2048t]   = (Wq^T x^T) with RoPE fused on PSUM->SBUF
  kT   [128d, 2g, 2048t]   same
  v    [128t, 16tt, 256gd] = x @ Wv (both groups side by side)
  S^T  [128s, 512t] PSUM   = kT_tile^T @ qT  (scores transposed)
  P    [128s, 512t] bf16   = exp(S^T/sqrt(D)) (no max-sub; |S|<~30 safe),
                             causal-zeroed via affine_select on diag tiles
  attnT[128d, 8h, 2048t]   = sum_s v^T P, normalized by 1/den via
                             ones-matmul denominator + partition_broadcast
  out  [128t, 2048e] fp32  = attnT^T @ Wo partial, accumulated over heads
"""

import numpy as np
import ml_dtypes

B, T, E = 4, 2048, 2048
G, QG, D = 4, 4, 128
HALF = D // 2
NCORES = 8
GPC = 2               # groups per core
HPC = GPC * QG        # heads per core = 8
TCH = 512             # t-chunk (moving free dim)
NTCH = T // TCH       # 4
NTT = T // 128        # 16 t-tiles
NET = E // 128        # 16 e-tiles
SCALE = 1.0 / float(np.sqrt(D))

_cached = {}


def _build():
    import concourse.bass as bass
    import concourse.mybir as mybir
    from concourse import bacc
    from concourse.tile import TileContext

    dt = mybir.dt
    nc = bacc.Bacc("TRN2", target_bir_lowering=False, debug=False,
                   num_devices=NCORES)

    xb = nc.dram_tensor("xb", [T, E], dt.bfloat16, kind="ExternalInput")
    wq = nc.dram_tensor("wq", [E, GPC * QG * D], dt.bfloat16, kind="ExternalInput")
    wk = nc.dram_tensor("wk", [E, GPC * D], dt.bfloat16, kind="ExternalInput")
    wv = nc.dram_tensor("wv", [E, GPC * D], dt.bfloat16, kind="ExternalInput")
    wo = nc.dram_tensor("wo", [GPC * QG * D, E], dt.bfloat16, kind="ExternalInput")
    cosT = nc.dram_tensor("cosT", [HALF, T], dt.float32, kind="ExternalInput")
    sinT = nc.dram_tensor("sinT", [HALF, T], dt.float32, kind="ExternalInput")
    out = nc.dram_tensor("out", [T, E], dt.float32, kind="ExternalOutput")

    with TileContext(nc) as tc:
        from contextlib import ExitStack
        with ExitStack() as outer:
            # long-lived tensors
            main = outer.enter_context(tc.tile_pool(name="main", bufs=1))
            qT = main.tile([128, HPC, T], dt.bfloat16, tag="qT")
            kT = main.tile([128, GPC, T], dt.bfloat16, tag="kT")
            vt = main.tile([128, NTT, GPC * D], dt.bfloat16, tag="vt")
            attnT = main.tile([128, HPC, T], dt.bfloat16, tag="attnT")
            ones = main.tile([128, 1], dt.float32, tag="ones")
            nc.gpsimd.memset(ones[:], 1.0)

            # ---------------- phase 1: QKV projections + rope ----------------
            with ExitStack() as ph1:
                wpool = ph1.enter_context(tc.tile_pool(name="wpool", bufs=1))
                xpool = ph1.enter_context(tc.tile_pool(name="xpool", bufs=2))
                cspool = ph1.enter_context(tc.tile_pool(name="cspool", bufs=1))
                rt = ph1.enter_context(tc.tile_pool(name="rt", bufs=2))
                pqk = ph1.enter_context(tc.tile_pool(name="pqk", bufs=3, space="PSUM"))
                pv = ph1.enter_context(tc.tile_pool(name="pv", bufs=2, space="PSUM"))

                twq = wpool.tile([128, NET, GPC * QG * D], dt.bfloat16, tag="twq")
                twk = wpool.tile([128, NET, GPC * D], dt.bfloat16, tag="twk")
                twv = wpool.tile([128, NET, GPC * D], dt.bfloat16, tag="twv")
                nc.sync.dma_start(out=twq[:], in_=wq.rearrange("(et p) j -> p et j", p=128))
                nc.sync.dma_start(out=twk[:], in_=wk.rearrange("(et p) j -> p et j", p=128))
                nc.sync.dma_start(out=twv[:], in_=wv.rearrange("(et p) j -> p et j", p=128))
                tcos = cspool.tile([HALF, T], dt.float32, tag="tcos")
                tsin = cspool.tile([HALF, T], dt.float32, tag="tsin")
                nc.sync.dma_start(out=tcos[:], in_=cosT[:])
                nc.sync.dma_start(out=tsin[:], in_=sinT[:])

                def rope_to(psum, dst_lo, dst_hi, tci):
                    cs = tcos[:, tci * TCH:(tci + 1) * TCH]
                    sn = tsin[:, tci * TCH:(tci + 1) * TCH]
                    t1 = rt.tile([HALF, TCH], dt.float32, tag="t1")
                    t2 = rt.tile([HALF, TCH], dt.float32, tag="t2")
                    nc.vector.tensor_mul(t1[:], psum[0:HALF, :], cs)
                    nc.vector.tensor_mul(t2[:], psum[HALF:D, :], sn)
                    nc.vector.tensor_sub(dst_lo, t1[:], t2[:])
                    t3 = rt.tile([HALF, TCH], dt.float32, tag="t1")
                    t4 = rt.tile([HALF, TCH], dt.float32, tag="t2")
                    nc.vector.tensor_mul(t3[:], psum[0:HALF, :], sn)
                    nc.vector.tensor_mul(t4[:], psum[HALF:D, :], cs)
                    nc.vector.tensor_add(dst_hi, t3[:], t4[:])

                for tci in range(NTCH):
                    xt = xpool.tile([128, NET, TCH], dt.bfloat16, tag="xt")
                    for e in range(NET):
                        nc.sync.dma_start_transpose(
                            out=xt[:, e, :],
                            in_=xb[tci * TCH:(tci + 1) * TCH, e * 128:(e + 1) * 128])
                    tsl = slice(tci * TCH, (tci + 1) * TCH)
                    for gi in range(GPC):
                        # k projection + rope
                        ps = pqk.tile([128, TCH], dt.float32, tag="pqk")
                        for e in range(NET):
                            nc.tensor.matmul(ps[:], twk[:, e, gi * D:(gi + 1) * D],
                                             xt[:, e, :], start=(e == 0), stop=(e == NET - 1))
                        rope_to(ps, kT[0:HALF, gi, tsl], kT[HALF:D, gi, tsl], tci)
                        # q projections + rope
                        for h in range(QG):
                            hh = gi * QG + h
                            ps = pqk.tile([128, TCH], dt.float32, tag="pqk")
                            col = gi * QG * D + h * D
                            for e in range(NET):
                                nc.tensor.matmul(ps[:], twq[:, e, col:col + D],
                                                 xt[:, e, :], start=(e == 0), stop=(e == NET - 1))
                            rope_to(ps, qT[0:HALF, hh, tsl], qT[HALF:D, hh, tsl], tci)
                    # v projection: out [128t, 256(g,d)] per t-subtile
                    for tl in range(TCH // 128):
                        tt = tci * (TCH // 128) + tl
                        psv = pv.tile([128, GPC * D], dt.float32, tag="pv")
                        for e in range(NET):
                            nc.tensor.matmul(psv[:], xt[:, e, tl * 128:(tl + 1) * 128],
                                             twv[:, e, :], start=(e == 0), stop=(e == NET - 1))
                        nc.scalar.copy(vt[:, tt, :], psv[:])

            # ---------------- phase 2: attention ----------------
            with ExitStack() as ph2:
                wopool = ph2.enter_context(tc.tile_pool(name="wopool", bufs=1))
                two = wopool.tile([128, HPC, E], dt.bfloat16, tag="two")
                nc.sync.dma_start(out=two[:], in_=wo.rearrange("(jt p) e -> p jt e", p=128))

                attn_pools = ph2.enter_context(ExitStack())
                pspool = attn_pools.enter_context(tc.tile_pool(name="pspool", bufs=3, space="PSUM"))
                pavpool = attn_pools.enter_context(tc.tile_pool(name="pavpool", bufs=2, space="PSUM"))
                pdenpool = attn_pools.enter_context(tc.tile_pool(name="pdenpool", bufs=2, space="PSUM"))
                ppool = attn_pools.enter_context(tc.tile_pool(name="ppool", bufs=6))
                denpool = attn_pools.enter_context(tc.tile_pool(name="denpool", bufs=2))
                rcpool = attn_pools.enter_context(tc.tile_pool(name="rcpool", bufs=2))

                for hh in range(HPC):
                    gi = hh // QG
                    for tci in range(NTCH):
                        tsl = slice(tci * TCH, (tci + 1) * TCH)
                        n_s = 4 * (tci + 1)
                        den = denpool.tile([128, TCH], dt.float32, tag="den")
                        pav = pavpool.tile([128, TCH], dt.float32, tag="pav")
                        deng = (hh * NTCH + tci) % 2
                        for st in range(n_s):
                            pss = pspool.tile([128, TCH], dt.float32, tag="pss")
                            nc.tensor.matmul(pss[:], kT[:, gi, st * 128:(st + 1) * 128],
                                             qT[:, hh, tsl], start=True, stop=True)
                            P = ppool.tile([128, TCH], dt.bfloat16, tag="P")
                            nc.scalar.activation(P[:], pss[:],
                                                 mybir.ActivationFunctionType.Exp,
                                                 scale=SCALE)
                            if st >= 4 * tci:  # diagonal tile: causal zeroing
                                nc.gpsimd.affine_select(
                                    out=P[:], in_=P[:], pattern=[[1, TCH]],
                                    compare_op=mybir.AluOpType.is_ge, fill=0.0,
                                    base=tci * TCH - st * 128, channel_multiplier=-1)
                            eng = nc.vector if deng == 0 else nc.gpsimd
                            if st == 0:
                                eng.tensor_copy(den[:], P[:])
                            else:
                                eng.tensor_add(den[:], den[:], P[:])
                            nc.tensor.matmul(pav[:], vt[:, st, gi * D:(gi + 1) * D],
                                             P[:], start=(st == 0), stop=(st == n_s - 1))
                        psd = pdenpool.tile([1, TCH], dt.float32, tag="psd")
                        nc.tensor.matmul(psd[:], ones[:], den[:], start=True, stop=True)
                        rcp = rcpool.tile([1, TCH], dt.float32, tag="rcp")
                        nc.vector.reciprocal(rcp[:], psd[:])
                        rcpB = rcpool.tile([128, TCH], dt.float32, tag="rcpB")
                        nc.gpsimd.partition_broadcast(rcpB[:], rcp[:])
                        nc.vector.tensor_mul(attnT[:, hh, tsl], pav[:], rcpB[:])

                attn_pools.close()

                # ---------------- phase 3: o_proj ----------------
                with ExitStack() as ph3:
                    opool = ph3.enter_context(tc.tile_pool(name="opool", bufs=2))
                    popool = ph3.enter_context(tc.tile_pool(name="popool", bufs=4, space="PSUM"))
                    for tt in range(NTT):
                        ost = opool.tile([128, E], dt.float32, tag="ost")
                        for ec in range(E // TCH):
                            po = popool.tile([128, TCH], dt.float32, tag="po")
                            for hh in range(HPC):
                                nc.tensor.matmul(po[:], attnT[:, hh, tt * 128:(tt + 1) * 128],
                                                 two[:, hh, ec * TCH:(ec + 1) * TCH],
                                                 start=(hh == 0), stop=(hh == HPC - 1))
                            nc.vector.tensor_copy(ost[:, ec * TCH:(ec + 1) * TCH], po[:])
                        nc.sync.dma_start(out=out[tt * 128:(tt + 1) * 128, :], in_=ost[:])

    nc.compile()
    return nc


def _get_nc():
    if "nc" not in _cached:
        _cached["nc"] = _build()
    return _cached["nc"]


def _make_in_maps(x, Wq, Wk, Wv, Wo):
    bf16 = ml_dtypes.bfloat16
    half = HALF
    inv_freq = 1.0 / (10000.0 ** (np.arange(half, dtype=np.float32) / half))
    ang = np.arange(T, dtype=np.float32)[:, None] * inv_freq[None, :]
    cosT = np.ascontiguousarray(np.cos(ang).T.astype(np.float32))
    sinT = np.ascontiguousarray(np.sin(ang).T.astype(np.float32))
    in_maps = []
    for c in range(NCORES):
        b = c // 2
        g0 = GPC * (c % 2)
        jlo, jhi = g0 * QG * D, (g0 + GPC) * QG * D
        klo, khi = g0 * D, (g0 + GPC) * D
        in_maps.append({
            "xb": np.ascontiguousarray(x[b]).astype(bf16),
            "wq": np.ascontiguousarray(Wq[:, jlo:jhi]).astype(bf16),
            "wk": np.ascontiguousarray(Wk[:, klo:khi]).astype(bf16),
            "wv": np.ascontiguousarray(Wv[:, klo:khi]).astype(bf16),
            "wo": np.ascontiguousarray(Wo[jlo:jhi, :]).astype(bf16),
            "cosT": cosT,
            "sinT": sinT,
        })
    return in_maps


def run(x, Wq, Wk, Wv, Wo, trace=False, **spmd_kwargs):
    from concourse.bass_utils import run_bass_kernel_spmd
    nc = _get_nc()
    in_maps = _make_in_maps(x, Wq, Wk, Wv, Wo)
    res = run_bass_kernel_spmd(nc, in_maps, list(range(NCORES)),
                               trace=trace, **spmd_kwargs)
    outp = np.empty((B, T, E), dtype=np.float32)
    for b in range(B):
        outp[b] = res.results[2 * b]["out"] + res.results[2 * b + 1]["out"]
    return outp, res


def kernel(x, Wq, Wk, Wv, Wo):
    outp, _ = run(np.asarray(x), np.asarray(Wq), np.asarray(Wk),
                  np.asarray(Wv), np.asarray(Wo))
    return outp


# revision 7
# speedup vs baseline: 1.1379x; 1.1379x over previous
"""GQA kernel for Trainium2, 8 NeuronCores.

Problem: B=4, T=2048, E=2048, G=4 kv groups, QG=4 queries/group, D=128.
Sharding: core c handles batch b=c//2 and kv-groups {2*(c%2), 2*(c%2)+1}.
Each core computes a partial o_proj output for its batch (its 2 groups'
contribution); host sums the two partials per batch.

Device layouts (all matmuls bf16 with fp32 PSUM accumulation):
  xT   [128e, 16et, 512t]  per t-chunk, via DMA-transpose of x[b] (bf16)
  qT   [128d, 8h, 2048t]   = (Wq^T x^T) with RoPE fused on PSUM->SBUF
  kT   [128d, 2g, 2048t]   same
  v    [128t, 16tt, 256gd] = x @ Wv (both groups side by side)
  S^T  [128s, 512t] PSUM   = kT_tile^T @ qT  (scores transposed)
  P    [128s, 512t] bf16   = exp(S^T/sqrt(D)) (no max-sub; |S|<~30 safe),
                             causal-zeroed via affine_select on diag tiles
  attnT[128d, 8h, 2048t]   = sum_s v^T P, normalized by 1/den via
                             ones-matmul denominator + partition_broadcast
  out  [128t, 2048e] fp32  = attnT^T @ Wo partial, accumulated over heads
"""

import numpy as np
import ml_dtypes

B, T, E = 4, 2048, 2048
G, QG, D = 4, 4, 128
HALF = D // 2
NCORES = 8
GPC = 2               # groups per core
HPC = GPC * QG        # heads per core = 8
TCH = 512             # t-chunk (moving free dim)
NTCH = T // TCH       # 4
NTT = T // 128        # 16 t-tiles
NET = E // 128        # 16 e-tiles
SCALE = 1.0 / float(np.sqrt(D))

_cached = {}


def _build():
    import concourse.bass as bass
    import concourse.mybir as mybir
    from concourse import bacc
    from concourse.tile import TileContext

    dt = mybir.dt
    nc = bacc.Bacc("TRN2", target_bir_lowering=False, debug=False,
                   num_devices=NCORES)

    xb = nc.dram_tensor("xb", [T, E], dt.bfloat16, kind="ExternalInput")
    wq = nc.dram_tensor("wq", [E, GPC * QG * D], dt.bfloat16, kind="ExternalInput")
    wk = nc.dram_tensor("wk", [E, GPC * D], dt.bfloat16, kind="ExternalInput")
    wv = nc.dram_tensor("wv", [E, GPC * D], dt.bfloat16, kind="ExternalInput")
    wo = nc.dram_tensor("wo", [GPC * QG * D, E], dt.bfloat16, kind="ExternalInput")
    cosT = nc.dram_tensor("cosT", [HALF, T], dt.float32, kind="ExternalInput")
    sinT = nc.dram_tensor("sinT", [HALF, T], dt.float32, kind="ExternalInput")
    out = nc.dram_tensor("out", [T, E], dt.float32, kind="ExternalOutput")

    with TileContext(nc) as tc:
        from contextlib import ExitStack
        with ExitStack() as outer:
            # long-lived tensors
            main = outer.enter_context(tc.tile_pool(name="main", bufs=1))
            qT = main.tile([128, HPC, T], dt.bfloat16, tag="qT")
            kT = main.tile([128, GPC, T], dt.bfloat16, tag="kT")
            vt = main.tile([128, NTT, GPC * D], dt.bfloat16, tag="vt")
            attnT = main.tile([128, HPC, T], dt.bfloat16, tag="attnT")
            ones = main.tile([128, 1], dt.float32, tag="ones")
            nc.gpsimd.memset(ones[:], 1.0)

            # ---------------- phase 1: QKV projections + rope ----------------
            with ExitStack() as ph1:
                wpool = ph1.enter_context(tc.tile_pool(name="wpool", bufs=1))
                xpool = ph1.enter_context(tc.tile_pool(name="xpool", bufs=2))
                cspool = ph1.enter_context(tc.tile_pool(name="cspool", bufs=1))
                rt = ph1.enter_context(tc.tile_pool(name="rt", bufs=2))
                pqk = ph1.enter_context(tc.tile_pool(name="pqk", bufs=3, space="PSUM"))
                pv = ph1.enter_context(tc.tile_pool(name="pv", bufs=2, space="PSUM"))

                twq = wpool.tile([128, NET, GPC * QG * D], dt.bfloat16, tag="twq")
                twk = wpool.tile([128, NET, GPC * D], dt.bfloat16, tag="twk")
                twv = wpool.tile([128, NET, GPC * D], dt.bfloat16, tag="twv")
                nc.sync.dma_start(out=twq[:], in_=wq.rearrange("(et p) j -> p et j", p=128))
                nc.sync.dma_start(out=twk[:], in_=wk.rearrange("(et p) j -> p et j", p=128))
                nc.sync.dma_start(out=twv[:], in_=wv.rearrange("(et p) j -> p et j", p=128))
                tcos = cspool.tile([HALF, T], dt.float32, tag="tcos")
                tsin = cspool.tile([HALF, T], dt.float32, tag="tsin")
                nc.sync.dma_start(out=tcos[:], in_=cosT[:])
                nc.sync.dma_start(out=tsin[:], in_=sinT[:])

                def rope_to(psum, dst_lo, dst_hi, tci):
                    cs = tcos[:, tci * TCH:(tci + 1) * TCH]
                    sn = tsin[:, tci * TCH:(tci + 1) * TCH]
                    t1 = rt.tile([HALF, TCH], dt.float32, tag="t1")
                    t2 = rt.tile([HALF, TCH], dt.float32, tag="t2")
                    nc.vector.tensor_mul(t1[:], psum[0:HALF, :], cs)
                    nc.vector.tensor_mul(t2[:], psum[HALF:D, :], sn)
                    nc.vector.tensor_sub(dst_lo, t1[:], t2[:])
                    t3 = rt.tile([HALF, TCH], dt.float32, tag="t1")
                    t4 = rt.tile([HALF, TCH], dt.float32, tag="t2")
                    nc.vector.tensor_mul(t3[:], psum[0:HALF, :], sn)
                    nc.vector.tensor_mul(t4[:], psum[HALF:D, :], cs)
                    nc.vector.tensor_add(dst_hi, t3[:], t4[:])

                for tci in range(NTCH):
                    xt = xpool.tile([128, NET, TCH], dt.bfloat16, tag="xt")
                    for e in range(NET):
                        nc.sync.dma_start_transpose(
                            out=xt[:, e, :],
                            in_=xb[tci * TCH:(tci + 1) * TCH, e * 128:(e + 1) * 128])
                    tsl = slice(tci * TCH, (tci + 1) * TCH)
                    for gi in range(GPC):
                        # k projection + rope
                        ps = pqk.tile([128, TCH], dt.float32, tag="pqk")
                        for e in range(NET):
                            nc.tensor.matmul(ps[:], twk[:, e, gi * D:(gi + 1) * D],
                                             xt[:, e, :], start=(e == 0), stop=(e == NET - 1))
                        rope_to(ps, kT[0:HALF, gi, tsl], kT[HALF:D, gi, tsl], tci)
                        # q projections + rope
                        for h in range(QG):
                            hh = gi * QG + h
                            ps = pqk.tile([128, TCH], dt.float32, tag="pqk")
                            col = gi * QG * D + h * D
                            for e in range(NET):
                                nc.tensor.matmul(ps[:], twq[:, e, col:col + D],
                                                 xt[:, e, :], start=(e == 0), stop=(e == NET - 1))
                            rope_to(ps, qT[0:HALF, hh, tsl], qT[HALF:D, hh, tsl], tci)
                    # v projection: out [128t, 256(g,d)] per t-subtile
                    for tl in range(TCH // 128):
                        tt = tci * (TCH // 128) + tl
                        psv = pv.tile([128, GPC * D], dt.float32, tag="pv")
                        for e in range(NET):
                            nc.tensor.matmul(psv[:], xt[:, e, tl * 128:(tl + 1) * 128],
                                             twv[:, e, :], start=(e == 0), stop=(e == NET - 1))
                        nc.scalar.copy(vt[:, tt, :], psv[:])

            # ------------- phase 2+3: attention + o_proj, chunk-pipelined -------------
            with ExitStack() as ph2:
                wopool = ph2.enter_context(tc.tile_pool(name="wopool", bufs=1))
                two = wopool.tile([128, HPC, E], dt.bfloat16, tag="two")
                nc.sync.dma_start(out=two[:], in_=wo.rearrange("(jt p) e -> p jt e", p=128))

                pspool = ph2.enter_context(tc.tile_pool(name="pspool", bufs=3, space="PSUM"))
                pavpool = ph2.enter_context(tc.tile_pool(name="pavpool", bufs=2, space="PSUM"))
                pdenpool = ph2.enter_context(tc.tile_pool(name="pdenpool", bufs=1, space="PSUM"))
                popool = ph2.enter_context(tc.tile_pool(name="popool", bufs=2, space="PSUM"))
                ppool = ph2.enter_context(tc.tile_pool(name="ppool", bufs=8))
                denpool = ph2.enter_context(tc.tile_pool(name="denpool", bufs=4))
                rcpool = ph2.enter_context(tc.tile_pool(name="rcpool", bufs=4))
                opool = ph2.enter_context(tc.tile_pool(name="opool", bufs=2))

                for tci in range(NTCH):
                    tsl = slice(tci * TCH, (tci + 1) * TCH)
                    n_s = 4 * (tci + 1)
                    for hh in range(HPC):
                        gi = hh // QG
                        den = denpool.tile([128, TCH], dt.float32, tag="den")
                        pav = pavpool.tile([128, TCH], dt.float32, tag="pav")
                        deng = (hh * NTCH + tci) % 2
                        for st in range(n_s):
                            pss = pspool.tile([128, TCH], dt.float32, tag="pss")
                            nc.tensor.matmul(pss[:], kT[:, gi, st * 128:(st + 1) * 128],
                                             qT[:, hh, tsl], start=True, stop=True)
                            P = ppool.tile([128, TCH], dt.bfloat16, tag="P")
                            nc.scalar.activation(P[:], pss[:],
                                                 mybir.ActivationFunctionType.Exp,
                                                 scale=SCALE)
                            if st >= 4 * tci:  # diagonal tile: causal zeroing
                                nc.gpsimd.affine_select(
                                    out=P[:], in_=P[:], pattern=[[1, TCH]],
                                    compare_op=mybir.AluOpType.is_ge, fill=0.0,
                                    base=tci * TCH - st * 128, channel_multiplier=-1)
                            eng = nc.vector if deng == 0 else nc.gpsimd
                            if st == 0:
                                eng.tensor_copy(den[:], P[:])
                            else:
                                eng.tensor_add(den[:], den[:], P[:])
                            nc.tensor.matmul(pav[:], vt[:, st, gi * D:(gi + 1) * D],
                                             P[:], start=(st == 0), stop=(st == n_s - 1))
                        psd = pdenpool.tile([1, TCH], dt.float32, tag="psd")
                        nc.tensor.matmul(psd[:], ones[:], den[:], start=True, stop=True)
                        rcp = rcpool.tile([1, TCH], dt.float32, tag="rcp")
                        nc.vector.reciprocal(rcp[:], psd[:])
                        rcpB = rcpool.tile([128, TCH], dt.float32, tag="rcpB")
                        nc.gpsimd.partition_broadcast(rcpB[:], rcp[:])
                        nc.vector.tensor_mul(attnT[:, hh, tsl], pav[:], rcpB[:])

                    # o_proj for this chunk's four t-tiles
                    for tl in range(TCH // 128):
                        tt = tci * (TCH // 128) + tl
                        ost = opool.tile([128, E], dt.float32, tag="ost")
                        for ec in range(E // TCH):
                            po = popool.tile([128, TCH], dt.float32, tag="po")
                            for hh in range(HPC):
                                nc.tensor.matmul(po[:], attnT[:, hh, tt * 128:(tt + 1) * 128],
                                                 two[:, hh, ec * TCH:(ec + 1) * TCH],
                                                 start=(hh == 0), stop=(hh == HPC - 1))
                            nc.vector.tensor_copy(ost[:, ec * TCH:(ec + 1) * TCH], po[:])
                        nc.sync.dma_start(out=out[tt * 128:(tt + 1) * 128, :], in_=ost[:])

    nc.compile()
    return nc


def _get_nc():
    if "nc" not in _cached:
        _cached["nc"] = _build()
    return _cached["nc"]


def _make_in_maps(x, Wq, Wk, Wv, Wo):
    bf16 = ml_dtypes.bfloat16
    half = HALF
    inv_freq = 1.0 / (10000.0 ** (np.arange(half, dtype=np.float32) / half))
    ang = np.arange(T, dtype=np.float32)[:, None] * inv_freq[None, :]
    cosT = np.ascontiguousarray(np.cos(ang).T.astype(np.float32))
    sinT = np.ascontiguousarray(np.sin(ang).T.astype(np.float32))
    in_maps = []
    for c in range(NCORES):
        b = c // 2
        g0 = GPC * (c % 2)
        jlo, jhi = g0 * QG * D, (g0 + GPC) * QG * D
        klo, khi = g0 * D, (g0 + GPC) * D
        in_maps.append({
            "xb": np.ascontiguousarray(x[b]).astype(bf16),
            "wq": np.ascontiguousarray(Wq[:, jlo:jhi]).astype(bf16),
            "wk": np.ascontiguousarray(Wk[:, klo:khi]).astype(bf16),
            "wv": np.ascontiguousarray(Wv[:, klo:khi]).astype(bf16),
            "wo": np.ascontiguousarray(Wo[jlo:jhi, :]).astype(bf16),
            "cosT": cosT,
            "sinT": sinT,
        })
    return in_maps


def run(x, Wq, Wk, Wv, Wo, trace=False, **spmd_kwargs):
    from concourse.bass_utils import run_bass_kernel_spmd
    nc = _get_nc()
    in_maps = _make_in_maps(x, Wq, Wk, Wv, Wo)
    res = run_bass_kernel_spmd(nc, in_maps, list(range(NCORES)),
                               trace=trace, **spmd_kwargs)
    outp = np.empty((B, T, E), dtype=np.float32)
    for b in range(B):
        outp[b] = res.results[2 * b]["out"] + res.results[2 * b + 1]["out"]
    return outp, res


def kernel(x, Wq, Wk, Wv, Wo):
    outp, _ = run(np.asarray(x), np.asarray(Wq), np.asarray(Wk),
                  np.asarray(Wv), np.asarray(Wo))
    return outp


# revision 9
# speedup vs baseline: 1.5630x; 1.3735x over previous
"""GQA kernel for Trainium2, 8 NeuronCores.

Problem: B=4, T=2048, E=2048, G=4 kv groups, QG=4 queries/group, D=128.
Sharding: core c handles batch b=c//2 and kv-groups {2*(c%2), 2*(c%2)+1}.
Each core computes a partial o_proj output for its batch (its 2 groups'
contribution); host sums the two partials per batch.

Device layouts (all matmuls bf16 with fp32 PSUM accumulation):
  xT   [128e, 16et, 512t]  per t-chunk, via DMA-transpose of x[b] (bf16)
  qT   [128d, 8h, 2048t]   = (Wq^T x^T) with RoPE fused on PSUM->SBUF
  kT   [128d, 2g, 2048t]   same
  v    [128t, 16tt, 256gd] = x @ Wv (both groups side by side)
  S^T  [128s, 512t] PSUM   = kT_tile^T @ qT  (scores transposed)
  P    [128s, 512t] bf16   = exp(S^T/sqrt(D)) (no max-sub; |S|<~30 safe),
                             diag tiles column-trimmed + triangle-zeroed
  den  [1, 512] PSUM       = ones^T @ P accumulated over s-tiles (PE, bf16)
  attnT[128d, 8h, 2048t]   = (sum_s v^T P) * (1/den) via partition_broadcast
  out  [128t, 2048e] fp32  = attnT^T @ Wo partial, accumulated over heads
"""

import numpy as np
import ml_dtypes

B, T, E = 4, 2048, 2048
G, QG, D = 4, 4, 128
HALF = D // 2
NCORES = 8
GPC = 2               # groups per core
HPC = GPC * QG        # heads per core = 8
TCH = 512             # t-chunk (moving free dim)
NTCH = T // TCH       # 4
NTT = T // 128        # 16 t-tiles
NET = E // 128        # 16 e-tiles
SCALE = 1.0 / float(np.sqrt(D))

_cached = {}


def _build():
    import concourse.bass as bass
    import concourse.mybir as mybir
    from concourse import bacc
    from concourse.tile import TileContext
    from contextlib import ExitStack

    dt = mybir.dt
    nc = bacc.Bacc("TRN2", target_bir_lowering=False, debug=False,
                   num_devices=NCORES)

    xb = nc.dram_tensor("xb", [T, E], dt.bfloat16, kind="ExternalInput")
    wq = nc.dram_tensor("wq", [E, GPC * QG * D], dt.bfloat16, kind="ExternalInput")
    wk = nc.dram_tensor("wk", [E, GPC * D], dt.bfloat16, kind="ExternalInput")
    wv = nc.dram_tensor("wv", [E, GPC * D], dt.bfloat16, kind="ExternalInput")
    wo = nc.dram_tensor("wo", [GPC * QG * D, E], dt.bfloat16, kind="ExternalInput")
    # stacked rope tables: cs2 = [cos;sin], sn2 = [sin;cos]  (128 x T)
    cs2d = nc.dram_tensor("cs2", [D, T], dt.float32, kind="ExternalInput")
    sn2d = nc.dram_tensor("sn2", [D, T], dt.float32, kind="ExternalInput")
    out = nc.dram_tensor("out", [T, E], dt.float32, kind="ExternalOutput")

    with TileContext(nc) as tc:
        with ExitStack() as outer:
            main = outer.enter_context(tc.tile_pool(name="main", bufs=1))
            qT = main.tile([128, HPC, T], dt.bfloat16, tag="qT")
            kT = main.tile([128, GPC, T], dt.bfloat16, tag="kT")
            vt = main.tile([128, NTT, GPC * D], dt.bfloat16, tag="vt")
            attnT = main.tile([128, HPC, T], dt.bfloat16, tag="attnT")
            onesb = main.tile([128, 1], dt.bfloat16, tag="onesb")
            nc.gpsimd.memset(onesb[:], 1.0)

            # ---------------- phase 1: QKV projections + rope ----------------
            with ExitStack() as ph1:
                wpool = ph1.enter_context(tc.tile_pool(name="wpool", bufs=1))
                xpool = ph1.enter_context(tc.tile_pool(name="xpool", bufs=2))
                cspool = ph1.enter_context(tc.tile_pool(name="cspool", bufs=1))
                rt = ph1.enter_context(tc.tile_pool(name="rt", bufs=3))
                pqk = ph1.enter_context(tc.tile_pool(name="pqk", bufs=3, space="PSUM"))
                pv = ph1.enter_context(tc.tile_pool(name="pv", bufs=2, space="PSUM"))

                twq = wpool.tile([128, NET, GPC * QG * D], dt.bfloat16, tag="twq")
                twk = wpool.tile([128, NET, GPC * D], dt.bfloat16, tag="twk")
                twv = wpool.tile([128, NET, GPC * D], dt.bfloat16, tag="twv")
                nc.sync.dma_start(out=twq[:], in_=wq.rearrange("(et p) j -> p et j", p=128))
                nc.sync.dma_start(out=twk[:], in_=wk.rearrange("(et p) j -> p et j", p=128))
                nc.sync.dma_start(out=twv[:], in_=wv.rearrange("(et p) j -> p et j", p=128))
                cs2 = cspool.tile([D, T], dt.float32, tag="cs2")
                sn2 = cspool.tile([D, T], dt.float32, tag="sn2")
                nc.sync.dma_start(out=cs2[:], in_=cs2d[:])
                nc.sync.dma_start(out=sn2[:], in_=sn2d[:])

                def rope_to(psum, dst_lo, dst_hi, tci):
                    csl = slice(tci * TCH, (tci + 1) * TCH)
                    cos = cs2[0:HALF, csl]
                    sin = sn2[0:HALF, csl]
                    t1 = rt.tile([HALF, TCH], dt.float32, tag="t1")
                    t2 = rt.tile([HALF, TCH], dt.float32, tag="t2")
                    nc.vector.tensor_mul(t1[:], psum[0:HALF, :], cos)
                    nc.vector.tensor_mul(t2[:], psum[HALF:D, :], sin)
                    nc.vector.tensor_sub(dst_lo, t1[:], t2[:])
                    t3 = rt.tile([HALF, TCH], dt.float32, tag="t1")
                    t4 = rt.tile([HALF, TCH], dt.float32, tag="t2")
                    nc.vector.tensor_mul(t3[:], psum[0:HALF, :], sin)
                    nc.vector.tensor_mul(t4[:], psum[HALF:D, :], cos)
                    nc.vector.tensor_add(dst_hi, t3[:], t4[:])

                for tci in range(NTCH):
                    xt = xpool.tile([128, NET, TCH], dt.bfloat16, tag="xt")
                    for e in range(NET):
                        nc.sync.dma_start_transpose(
                            out=xt[:, e, :],
                            in_=xb[tci * TCH:(tci + 1) * TCH, e * 128:(e + 1) * 128])
                    tsl = slice(tci * TCH, (tci + 1) * TCH)
                    for gi in range(GPC):
                        ps = pqk.tile([128, TCH], dt.float32, tag="pqk")
                        for e in range(NET):
                            nc.tensor.matmul(ps[:], twk[:, e, gi * D:(gi + 1) * D],
                                             xt[:, e, :], start=(e == 0), stop=(e == NET - 1))
                        rope_to(ps, kT[0:HALF, gi, tsl], kT[HALF:D, gi, tsl], tci)
                        for h in range(QG):
                            hh = gi * QG + h
                            ps = pqk.tile([128, TCH], dt.float32, tag="pqk")
                            col = gi * QG * D + h * D
                            for e in range(NET):
                                nc.tensor.matmul(ps[:], twq[:, e, col:col + D],
                                                 xt[:, e, :], start=(e == 0), stop=(e == NET - 1))
                            rope_to(ps, qT[0:HALF, hh, tsl], qT[HALF:D, hh, tsl], tci)
                    for tl in range(TCH // 128):
                        tt = tci * (TCH // 128) + tl
                        psv = pv.tile([128, GPC * D], dt.float32, tag="pv")
                        for e in range(NET):
                            nc.tensor.matmul(psv[:], xt[:, e, tl * 128:(tl + 1) * 128],
                                             twv[:, e, :], start=(e == 0), stop=(e == NET - 1))
                        nc.scalar.copy(vt[:, tt, :], psv[:])

            # ------------- phase 2+3: attention + o_proj, chunk-pipelined -------------
            with ExitStack() as ph2:
                wopool = ph2.enter_context(tc.tile_pool(name="wopool", bufs=1))
                two = wopool.tile([128, HPC, E], dt.bfloat16, tag="two")
                nc.sync.dma_start(out=two[:], in_=wo.rearrange("(jt p) e -> p jt e", p=128))

                pspool = ph2.enter_context(tc.tile_pool(name="pspool", bufs=2, space="PSUM"))
                pavpool = ph2.enter_context(tc.tile_pool(name="pavpool", bufs=2, space="PSUM"))
                pdenpool = ph2.enter_context(tc.tile_pool(name="pdenpool", bufs=2, space="PSUM"))
                popool = ph2.enter_context(tc.tile_pool(name="popool", bufs=2, space="PSUM"))
                ppool = ph2.enter_context(tc.tile_pool(name="ppool", bufs=8))
                rcpool = ph2.enter_context(tc.tile_pool(name="rcpool", bufs=4))
                opool = ph2.enter_context(tc.tile_pool(name="opool", bufs=2))

                for tci in range(NTCH):
                    tsl = slice(tci * TCH, (tci + 1) * TCH)
                    n_s = 4 * (tci + 1)
                    for hh in range(HPC):
                        gi = hh // QG
                        pav = pavpool.tile([128, TCH], dt.float32, tag="pav")
                        psd = pdenpool.tile([1, TCH], dt.float32, tag="psd")
                        for st in range(n_s):
                            di = st - 4 * tci           # >=0 on diagonal tiles
                            c0 = 128 * di if di > 0 else 0
                            first, last = (st == 0), (st == n_s - 1)
                            pss = pspool.tile([128, TCH], dt.float32, tag="pss")
                            nc.tensor.matmul(pss[:, c0:], kT[:, gi, st * 128:(st + 1) * 128],
                                             qT[:, hh, tci * TCH + c0:(tci + 1) * TCH],
                                             start=True, stop=True)
                            P = ppool.tile([128, TCH], dt.bfloat16, tag="P")
                            nc.scalar.activation(P[:, c0:], pss[:, c0:],
                                                 mybir.ActivationFunctionType.Exp,
                                                 scale=SCALE)
                            if di >= 0:  # zero the triangle block [c0, c0+128)
                                nc.gpsimd.affine_select(
                                    out=P[:, c0:c0 + 128], in_=P[:, c0:c0 + 128],
                                    pattern=[[1, 128]],
                                    compare_op=mybir.AluOpType.is_ge, fill=0.0,
                                    base=0, channel_multiplier=-1)
                            nc.tensor.matmul(psd[:, c0:], onesb[:], P[:, c0:],
                                             start=first, stop=last)
                            nc.tensor.matmul(pav[:, c0:], vt[:, st, gi * D:(gi + 1) * D],
                                             P[:, c0:], start=first, stop=last)
                        rcp = rcpool.tile([1, TCH], dt.float32, tag="rcp")
                        nc.vector.reciprocal(rcp[:], psd[:])
                        rcpB = rcpool.tile([128, TCH], dt.float32, tag="rcpB")
                        nc.gpsimd.partition_broadcast(rcpB[:], rcp[:])
                        nc.vector.tensor_mul(attnT[:, hh, tsl], pav[:], rcpB[:])

                    # o_proj for this chunk's four t-tiles
                    for tl in range(TCH // 128):
                        tt = tci * (TCH // 128) + tl
                        ost = opool.tile([128, E], dt.float32, tag="ost")
                        for ec in range(E // TCH):
                            po = popool.tile([128, TCH], dt.float32, tag="po")
                            for hh in range(HPC):
                                nc.tensor.matmul(po[:], attnT[:, hh, tt * 128:(tt + 1) * 128],
                                                 two[:, hh, ec * TCH:(ec + 1) * TCH],
                                                 start=(hh == 0), stop=(hh == HPC - 1))
                            nc.scalar.copy(ost[:, ec * TCH:(ec + 1) * TCH], po[:])
                        nc.sync.dma_start(out=out[tt * 128:(tt + 1) * 128, :], in_=ost[:])

    nc.compile()
    return nc


def _get_nc():
    if "nc" not in _cached:
        _cached["nc"] = _build()
    return _cached["nc"]


def _make_in_maps(x, Wq, Wk, Wv, Wo):
    bf16 = ml_dtypes.bfloat16
    half = HALF
    inv_freq = 1.0 / (10000.0 ** (np.arange(half, dtype=np.float32) / half))
    ang = np.arange(T, dtype=np.float32)[:, None] * inv_freq[None, :]
    cosT = np.cos(ang).T.astype(np.float32)   # [64, T]
    sinT = np.sin(ang).T.astype(np.float32)
    cs2 = np.ascontiguousarray(np.concatenate([cosT, sinT], axis=0))  # [128, T]
    sn2 = np.ascontiguousarray(np.concatenate([sinT, cosT], axis=0))
    in_maps = []
    for c in range(NCORES):
        b = c // 2
        g0 = GPC * (c % 2)
        jlo, jhi = g0 * QG * D, (g0 + GPC) * QG * D
        klo, khi = g0 * D, (g0 + GPC) * D
        in_maps.append({
            "xb": np.ascontiguousarray(x[b]).astype(bf16),
            "wq": np.ascontiguousarray(Wq[:, jlo:jhi]).astype(bf16),
            "wk": np.ascontiguousarray(Wk[:, klo:khi]).astype(bf16),
            "wv": np.ascontiguousarray(Wv[:, klo:khi]).astype(bf16),
            "wo": np.ascontiguousarray(Wo[jlo:jhi, :]).astype(bf16),
            "cs2": cs2,
            "sn2": sn2,
        })
    return in_maps


def run(x, Wq, Wk, Wv, Wo, trace=False, **spmd_kwargs):
    from concourse.bass_utils import run_bass_kernel_spmd
    nc = _get_nc()
    in_maps = _make_in_maps(x, Wq, Wk, Wv, Wo)
    res = run_bass_kernel_spmd(nc, in_maps, list(range(NCORES)),
                               trace=trace, **spmd_kwargs)
    outp = np.empty((B, T, E), dtype=np.float32)
    for b in range(B):
        outp[b] = res.results[2 * b]["out"] + res.results[2 * b + 1]["out"]
    return outp, res


def kernel(x, Wq, Wk, Wv, Wo):
    outp, _ = run(np.asarray(x), np.asarray(Wq), np.asarray(Wk),
                  np.asarray(Wv), np.asarray(Wo))
    return outp


# revision 11
# speedup vs baseline: 1.7384x; 1.1123x over previous
"""GQA kernel for Trainium2, 8 NeuronCores.

Problem: B=4, T=2048, E=2048, G=4 kv groups, QG=4 queries/group, D=128.
Sharding: core c handles batch b=c//2 and kv-groups {2*(c%2), 2*(c%2)+1}.
Each core computes a partial o_proj output for its batch (its 2 groups'
contribution); host sums the two partials per batch.

Device layouts (all matmuls bf16 with fp32 PSUM accumulation):
  xT   [128e, 16et, 512t]  per t-chunk, via DMA-transpose of x[b] (bf16)
  qT   [128d, 8h, 2048t]   = (Wq^T x^T) with RoPE fused on PSUM->SBUF
  kT   [128d, 2g, 2048t]   same
  v    [128t, 16tt, 256gd] = x @ Wv (both groups side by side)
  S^T  [128s, 512t] PSUM   = kT_tile^T @ qT  (scores transposed)
  P    [128s, 512t] bf16   = exp(S^T/sqrt(D)) (no max-sub; |S|<~30 safe),
                             diag tiles column-trimmed + triangle-zeroed
  den  [1, 512] PSUM       = ones^T @ P accumulated over s-tiles (PE, bf16)
  attnT[128d, 8h, 2048t]   = (sum_s v^T P) * (1/den) via partition_broadcast
  out  [128t, 2048e] fp32  = attnT^T @ Wo partial, accumulated over heads
"""

import numpy as np
import ml_dtypes

B, T, E = 4, 2048, 2048
G, QG, D = 4, 4, 128
HALF = D // 2
NCORES = 8
GPC = 2               # groups per core
HPC = GPC * QG        # heads per core = 8
TCH = 512             # t-chunk (moving free dim)
NTCH = T // TCH       # 4
NTT = T // 128        # 16 t-tiles
NET = E // 128        # 16 e-tiles
SCALE = 1.0 / float(np.sqrt(D))

_cached = {}


def _build():
    import concourse.bass as bass
    import concourse.mybir as mybir
    from concourse import bacc
    from concourse.tile import TileContext
    from contextlib import ExitStack

    dt = mybir.dt
    nc = bacc.Bacc("TRN2", target_bir_lowering=False, debug=False,
                   num_devices=NCORES)

    xb = nc.dram_tensor("xb", [T, E], dt.bfloat16, kind="ExternalInput")
    wq = nc.dram_tensor("wq", [E, GPC * QG * D], dt.bfloat16, kind="ExternalInput")
    wk = nc.dram_tensor("wk", [E, GPC * D], dt.bfloat16, kind="ExternalInput")
    wv = nc.dram_tensor("wv", [E, GPC * D], dt.bfloat16, kind="ExternalInput")
    wo = nc.dram_tensor("wo", [GPC * QG * D, E], dt.bfloat16, kind="ExternalInput")
    # stacked rope tables: cs2 = [cos;sin], sn2 = [sin;cos]  (128 x T)
    cs2d = nc.dram_tensor("cs2", [D, T], dt.float32, kind="ExternalInput")
    sn2d = nc.dram_tensor("sn2", [D, T], dt.float32, kind="ExternalInput")
    out = nc.dram_tensor("out", [T, E], dt.float32, kind="ExternalOutput")

    with TileContext(nc) as tc:
        with ExitStack() as outer:
            main = outer.enter_context(tc.tile_pool(name="main", bufs=1))
            qT = main.tile([128, HPC, T], dt.bfloat16, tag="qT")
            kT = main.tile([128, GPC, T], dt.bfloat16, tag="kT")
            vt = main.tile([128, NTT, GPC * D], dt.bfloat16, tag="vt")
            attnT = main.tile([128, HPC, T], dt.bfloat16, tag="attnT")
            onesb = main.tile([128, 128], dt.bfloat16, tag="onesb")
            nc.gpsimd.memset(onesb[:], 1.0)

            # ---------------- phase 1: QKV projections + rope ----------------
            with ExitStack() as ph1:
                wpool = ph1.enter_context(tc.tile_pool(name="wpool", bufs=1))
                xpool = ph1.enter_context(tc.tile_pool(name="xpool", bufs=2))
                cspool = ph1.enter_context(tc.tile_pool(name="cspool", bufs=1))
                rt = ph1.enter_context(tc.tile_pool(name="rt", bufs=3))
                pqk = ph1.enter_context(tc.tile_pool(name="pqk", bufs=3, space="PSUM"))
                pv = ph1.enter_context(tc.tile_pool(name="pv", bufs=2, space="PSUM"))

                twq = wpool.tile([128, NET, GPC * QG * D], dt.bfloat16, tag="twq")
                twk = wpool.tile([128, NET, GPC * D], dt.bfloat16, tag="twk")
                twv = wpool.tile([128, NET, GPC * D], dt.bfloat16, tag="twv")
                nc.sync.dma_start(out=twq[:], in_=wq.rearrange("(et p) j -> p et j", p=128))
                nc.sync.dma_start(out=twk[:], in_=wk.rearrange("(et p) j -> p et j", p=128))
                nc.sync.dma_start(out=twv[:], in_=wv.rearrange("(et p) j -> p et j", p=128))
                cs2 = cspool.tile([D, T], dt.float32, tag="cs2")
                sn2 = cspool.tile([D, T], dt.float32, tag="sn2")
                nc.sync.dma_start(out=cs2[:], in_=cs2d[:])
                nc.sync.dma_start(out=sn2[:], in_=sn2d[:])

                def rope_to(psum, dst_lo, dst_hi, tci):
                    csl = slice(tci * TCH, (tci + 1) * TCH)
                    cos = cs2[0:HALF, csl]
                    sin = sn2[0:HALF, csl]
                    t1 = rt.tile([HALF, TCH], dt.float32, tag="t1")
                    t2 = rt.tile([HALF, TCH], dt.float32, tag="t2")
                    nc.vector.tensor_mul(t1[:], psum[0:HALF, :], cos)
                    nc.vector.tensor_mul(t2[:], psum[HALF:D, :], sin)
                    nc.vector.tensor_sub(dst_lo, t1[:], t2[:])
                    t3 = rt.tile([HALF, TCH], dt.float32, tag="t1")
                    t4 = rt.tile([HALF, TCH], dt.float32, tag="t2")
                    nc.vector.tensor_mul(t3[:], psum[0:HALF, :], sin)
                    nc.vector.tensor_mul(t4[:], psum[HALF:D, :], cos)
                    nc.vector.tensor_add(dst_hi, t3[:], t4[:])

                for tci in range(NTCH):
                    xt = xpool.tile([128, NET, TCH], dt.bfloat16, tag="xt")
                    nc.sync.dma_start_transpose(
                        out=xt[:], in_=xb[tci * TCH:(tci + 1) * TCH, :])
                    tsl = slice(tci * TCH, (tci + 1) * TCH)
                    for gi in range(GPC):
                        ps = pqk.tile([128, TCH], dt.float32, tag="pqk")
                        for e in range(NET):
                            nc.tensor.matmul(ps[:], twk[:, e, gi * D:(gi + 1) * D],
                                             xt[:, e, :], start=(e == 0), stop=(e == NET - 1))
                        rope_to(ps, kT[0:HALF, gi, tsl], kT[HALF:D, gi, tsl], tci)
                        for h in range(QG):
                            hh = gi * QG + h
                            ps = pqk.tile([128, TCH], dt.float32, tag="pqk")
                            col = gi * QG * D + h * D
                            for e in range(NET):
                                nc.tensor.matmul(ps[:], twq[:, e, col:col + D],
                                                 xt[:, e, :], start=(e == 0), stop=(e == NET - 1))
                            rope_to(ps, qT[0:HALF, hh, tsl], qT[HALF:D, hh, tsl], tci)
                    for tl in range(TCH // 128):
                        tt = tci * (TCH // 128) + tl
                        psv = pv.tile([128, GPC * D], dt.float32, tag="pv")
                        for e in range(NET):
                            nc.tensor.matmul(psv[:], xt[:, e, tl * 128:(tl + 1) * 128],
                                             twv[:, e, :], start=(e == 0), stop=(e == NET - 1))
                        nc.scalar.copy(vt[:, tt, :], psv[:])

            # ------------- phase 2+3: attention + o_proj, chunk-pipelined -------------
            with ExitStack() as ph2:
                wopool = ph2.enter_context(tc.tile_pool(name="wopool", bufs=1))
                two = wopool.tile([128, HPC, E], dt.bfloat16, tag="two")
                nc.sync.dma_start(out=two[:], in_=wo.rearrange("(jt p) e -> p jt e", p=128))

                pspool = ph2.enter_context(tc.tile_pool(name="pspool", bufs=2, space="PSUM"))
                pavpool = ph2.enter_context(tc.tile_pool(name="pavpool", bufs=2, space="PSUM"))
                pdenpool = ph2.enter_context(tc.tile_pool(name="pdenpool", bufs=2, space="PSUM"))
                popool = ph2.enter_context(tc.tile_pool(name="popool", bufs=2, space="PSUM"))
                ppool = ph2.enter_context(tc.tile_pool(name="ppool", bufs=8))
                rcpool = ph2.enter_context(tc.tile_pool(name="rcpool", bufs=4))
                opool = ph2.enter_context(tc.tile_pool(name="opool", bufs=2))

                for tci in range(NTCH):
                    tsl = slice(tci * TCH, (tci + 1) * TCH)
                    n_s = 4 * (tci + 1)
                    for hh in range(HPC):
                        gi = hh // QG
                        pav = pavpool.tile([128, TCH], dt.float32, tag="pav")
                        psd = pdenpool.tile([128, TCH], dt.float32, tag="psd")
                        for st in range(n_s):
                            di = st - 4 * tci           # >=0 on diagonal tiles
                            c0 = 128 * di if di > 0 else 0
                            first, last = (st == 0), (st == n_s - 1)
                            pss = pspool.tile([128, TCH], dt.float32, tag="pss")
                            nc.tensor.matmul(pss[:, c0:], kT[:, gi, st * 128:(st + 1) * 128],
                                             qT[:, hh, tci * TCH + c0:(tci + 1) * TCH],
                                             start=True, stop=True)
                            P = ppool.tile([128, TCH], dt.bfloat16, tag="P")
                            nc.scalar.activation(P[:, c0:], pss[:, c0:],
                                                 mybir.ActivationFunctionType.Exp,
                                                 scale=SCALE)
                            if di >= 0:  # zero the triangle block [c0, c0+128)
                                nc.gpsimd.affine_select(
                                    out=P[:, c0:c0 + 128], in_=P[:, c0:c0 + 128],
                                    pattern=[[1, 128]],
                                    compare_op=mybir.AluOpType.is_ge, fill=0.0,
                                    base=0, channel_multiplier=-1)
                            nc.tensor.matmul(psd[:, c0:], onesb[:], P[:, c0:],
                                             start=first, stop=last)
                            nc.tensor.matmul(pav[:, c0:], vt[:, st, gi * D:(gi + 1) * D],
                                             P[:, c0:], start=first, stop=last)
                        rcpB = rcpool.tile([128, TCH], dt.float32, tag="rcpB")
                        nc.vector.reciprocal(rcpB[:], psd[:])
                        nc.vector.tensor_mul(attnT[:, hh, tsl], pav[:], rcpB[:])

                    # o_proj for this chunk's four t-tiles
                    for tl in range(TCH // 128):
                        tt = tci * (TCH // 128) + tl
                        ost = opool.tile([128, E], dt.float32, tag="ost")
                        for ec in range(E // TCH):
                            po = popool.tile([128, TCH], dt.float32, tag="po")
                            for hh in range(HPC):
                                nc.tensor.matmul(po[:], attnT[:, hh, tt * 128:(tt + 1) * 128],
                                                 two[:, hh, ec * TCH:(ec + 1) * TCH],
                                                 start=(hh == 0), stop=(hh == HPC - 1))
                            nc.scalar.copy(ost[:, ec * TCH:(ec + 1) * TCH], po[:])
                        nc.sync.dma_start(out=out[tt * 128:(tt + 1) * 128, :], in_=ost[:])

    nc.compile()
    return nc


def _get_nc():
    if "nc" not in _cached:
        _cached["nc"] = _build()
    return _cached["nc"]


def _make_in_maps(x, Wq, Wk, Wv, Wo):
    bf16 = ml_dtypes.bfloat16
    half = HALF
    inv_freq = 1.0 / (10000.0 ** (np.arange(half, dtype=np.float32) / half))
    ang = np.arange(T, dtype=np.float32)[:, None] * inv_freq[None, :]
    cosT = np.cos(ang).T.astype(np.float32)   # [64, T]
    sinT = np.sin(ang).T.astype(np.float32)
    cs2 = np.ascontiguousarray(np.concatenate([cosT, sinT], axis=0))  # [128, T]
    sn2 = np.ascontiguousarray(np.concatenate([sinT, cosT], axis=0))
    in_maps = []
    for c in range(NCORES):
        b = c // 2
        g0 = GPC * (c % 2)
        jlo, jhi = g0 * QG * D, (g0 + GPC) * QG * D
        klo, khi = g0 * D, (g0 + GPC) * D
        in_maps.append({
            "xb": np.ascontiguousarray(x[b]).astype(bf16),
            "wq": np.ascontiguousarray(Wq[:, jlo:jhi]).astype(bf16),
            "wk": np.ascontiguousarray(Wk[:, klo:khi]).astype(bf16),
            "wv": np.ascontiguousarray(Wv[:, klo:khi]).astype(bf16),
            "wo": np.ascontiguousarray(Wo[jlo:jhi, :]).astype(bf16),
            "cs2": cs2,
            "sn2": sn2,
        })
    return in_maps


def run(x, Wq, Wk, Wv, Wo, trace=False, **spmd_kwargs):
    from concourse.bass_utils import run_bass_kernel_spmd
    nc = _get_nc()
    in_maps = _make_in_maps(x, Wq, Wk, Wv, Wo)
    res = run_bass_kernel_spmd(nc, in_maps, list(range(NCORES)),
                               trace=trace, **spmd_kwargs)
    outp = np.empty((B, T, E), dtype=np.float32)
    for b in range(B):
        outp[b] = res.results[2 * b]["out"] + res.results[2 * b + 1]["out"]
    return outp, res


def kernel(x, Wq, Wk, Wv, Wo):
    outp, _ = run(np.asarray(x), np.asarray(Wq), np.asarray(Wk),
                  np.asarray(Wv), np.asarray(Wo))
    return outp
